# revision 12
# baseline (speedup 1.0000x reference)
"""Trainium2 Bass kernel for nn_GNN_layer (gnn_message_passing).

Host contract: kernel(**inputs) takes the FULL unsharded inputs (as produced
by setup_inputs) and returns the full (A_t, out) tuple, running the Bass
kernel data-parallel over the graph batch dim on 8 NeuronCores.

Device strategy per core (2 graphs):
  - load A row-slabs [128,1024]; fused DVE prescale (c0*A, in place) with
    free-axis accum -> c0*rowsum; fused mult-with-identity reduce -> diag
  - PE block-transposes of c0*A (batched 4 blocks/PSUM bank, ACT copy out)
  - stats cascade -> u = c3*rowmean + c4*diag + p1, s = c1*mean_all +
    c2*mean_diag + p2 (p1/p2/meanX via tiny PE/DVE ops on X)
  - A_t = (c0*A + (u_i+s)) + u_j : one fused DVE scalar_tensor_tensor per slab
  - out = (A_t@X2_t)/n + X1_t via decomposition:
      A_t@X2 = c0*(A@X2) + u (x) colsum2 + 1 (x) (u^T X2 + s*colsum2)
    PE does c0A@X2 with transposed blocks; the rank-1 terms are K=1 matmuls
    accumulated into the same PSUM group.
"""

import os
import sys

import numpy as np

for _p in ("/opt/trn_rl_repo", os.path.expanduser("~/.axon_site/_ro/trn_rl_repo")):
    if os.path.isdir(_p) and _p not in sys.path:
        sys.path.insert(0, _p)

N_FULL, n, D = 16, 1024, 64
NCORES = 8
G = N_FULL // NCORES  # graphs per core
P = 128
NT = n // P  # 8 row tiles per graph

_BUILD_CACHE = {}


def _build(prescale_is_c0: bool, stage: int = 99):
    """Build the per-core Bass program. prescale_is_c0: if True the slab
    prescale multiplies by c0 (dc[0]) and rowmean is recovered via 1/(c0*n);
    if False the prescale is by 1.0 (used when |c0| is tiny)."""
    key = (prescale_is_c0, stage)
    if key in _BUILD_CACHE:
        return _BUILD_CACHE[key]

    import concourse.bass as bass
    import concourse.mybir as mybir
    from concourse import bacc, tile
    from concourse.masks import make_identity

    fp32 = mybir.dt.float32
    Alu = mybir.AluOpType
    Act = mybir.ActivationFunctionType
    AxisX = mybir.AxisListType.X
    from concourse import bass_isa

    nc = bacc.Bacc("TRN2", target_bir_lowering=False)

    A = nc.dram_tensor("A", [G, n, n], fp32, kind="ExternalInput")
    X = nc.dram_tensor("X", [G, n, D], fp32, kind="ExternalInput")
    dc = nc.dram_tensor("dc", [128, 8], fp32, kind="ExternalInput")
    wA1r = nc.dram_tensor("wA1r", [128, D], fp32, kind="ExternalInput")
    wA2c = nc.dram_tensor("wA2c", [D, 1], fp32, kind="ExternalInput")
    w11T = nc.dram_tensor("w11T", [D, D], fp32, kind="ExternalInput")
    w12T = nc.dram_tensor("w12T", [D, D], fp32, kind="ExternalInput")
    w21T = nc.dram_tensor("w21T", [D, D], fp32, kind="ExternalInput")
    w22T = nc.dram_tensor("w22T", [D, D], fp32, kind="ExternalInput")
    w13r = nc.dram_tensor("w13r", [1, D], fp32, kind="ExternalInput")
    w14r = nc.dram_tensor("w14r", [1, D], fp32, kind="ExternalInput")
    w23r = nc.dram_tensor("w23r", [1, D], fp32, kind="ExternalInput")
    w24r = nc.dram_tensor("w24r", [1, D], fp32, kind="ExternalInput")
    w15c = nc.dram_tensor("w15c", [D, 1], fp32, kind="ExternalInput")
    w16c = nc.dram_tensor("w16c", [D, 1], fp32, kind="ExternalInput")
    w25c = nc.dram_tensor("w25c", [D, 1], fp32, kind="ExternalInput")
    w26c = nc.dram_tensor("w26c", [D, 1], fp32, kind="ExternalInput")

    At = nc.dram_tensor("At", [G, n, n], fp32, kind="ExternalOutput")
    Out = nc.dram_tensor("Out", [G, n, D], fp32, kind="ExternalOutput")

    with tile.TileContext(nc) as tc:
        with (
            tc.tile_pool(name="const", bufs=1) as cpool,
            tc.tile_pool(name="slab", bufs=2 * NT) as spool,
            tc.tile_pool(name="atT", bufs=2 * NT) as tpool,
            tc.tile_pool(name="xbuf", bufs=2) as xpool,
            tc.tile_pool(name="small", bufs=2) as smpool,
            tc.tile_pool(name="rows", bufs=2) as rpool,
            tc.tile_pool(name="urep", bufs=2) as upool,
            tc.tile_pool(name="obuf", bufs=2) as opool,
            tc.tile_pool(name="ps_t", bufs=2, space="PSUM") as ps_t,
            tc.tile_pool(name="ps_mm", bufs=2, space="PSUM") as ps_mm,
            tc.tile_pool(name="ps_x", bufs=2, space="PSUM") as ps_x,
            tc.tile_pool(name="ps_s1", bufs=1, space="PSUM") as ps_s1,
            tc.tile_pool(name="ps_s2", bufs=1, space="PSUM") as ps_s2,
        ):
            # ---------------- one-time constants ----------------
            ident = cpool.tile([P, P], fp32, tag="ident")
            make_identity(nc, ident[:])
            ones_col = cpool.tile([P, 1], fp32, tag="ones_col")
            nc.vector.memset(ones_col[:], 1.0)
            ones_row = cpool.tile([1, P], fp32, tag="ones_row")
            nc.vector.memset(ones_row[:], 1.0)

            dc_rep = cpool.tile([P, 8], fp32, tag="dc_rep")
            nc.sync.dma_start(dc_rep[:], dc[:])
            ap_q = dc_rep[:, 0:1]        # prescale factor q (c0 or 1)
            ap_rinv = dc_rep[:, 1:2]     # 1/(q*n)
            ap_c3 = dc_rep[:, 2:3]
            ap_c4 = dc_rep[:, 3:4]
            ap_c1 = dc_rep[:, 4:5]       # c1
            ap_c2 = dc_rep[:, 5:6]       # c2
            ap_evs = dc_rep[:, 6:7]      # c0/(q*n)  (eviction scale)
            ap_qc0 = dc_rep[0:1, 7:8]    # q/c0      (rank-1 rhs fixup)

            wA1_rep = cpool.tile([P, D], fp32, tag="wA1_rep")
            nc.sync.dma_start(wA1_rep[:], wA1r[:])

            wA2_sb = cpool.tile([D, 1], fp32, tag="wA2_sb")
            nc.sync.dma_start(wA2_sb[:], wA2c[:])
            wT_sb = {}
            for nm, t_ in (("w11T", w11T), ("w12T", w12T), ("w21T", w21T), ("w22T", w22T)):
                s_ = cpool.tile([D, D], fp32, tag=nm)
                nc.sync.dma_start(s_[:], t_[:])
                wT_sb[nm] = s_
            wr_sb = {}
            for nm, t_ in (("w13r", w13r), ("w14r", w14r), ("w23r", w23r), ("w24r", w24r)):
                s_ = cpool.tile([1, D], fp32, tag=nm)
                nc.sync.dma_start(s_[:], t_[:])
                wr_sb[nm] = s_
            wc_sb = {}
            for nm, t_ in (("w15c", w15c), ("w16c", w16c), ("w25c", w25c), ("w26c", w26c)):
                s_ = cpool.tile([D, 1], fp32, tag=nm)
                nc.sync.dma_start(s_[:], t_[:])
                wc_sb[nm] = s_

            # PE sync pre-warm: first PE instruction depends only on ident
            # (Pool sem) so later matmuls carry at most one new sync wait.
            dummy_ps = ps_s1.tile([P, P], fp32, tag="small64")
            nc.tensor.transpose(dummy_ps[:], ident[:], ident[:])

            # ---------------- per-graph program ----------------
            for g in range(G):
                # ---- X load, transposes, Xsum, p1 ----
                Xg = xpool.tile([P, NT, D], fp32, tag="Xg")
                nc.sync.dma_start(Xg[:], X[g].rearrange("(t p) d -> p t d", p=P))

                XTg = xpool.tile([D, NT, P], fp32, tag="XTg")
                for t in range(NT):
                    pxt = ps_x.tile([D, P], fp32, tag="ps_x")
                    nc.tensor.transpose(pxt[:], Xg[:, t, :], ident[:])
                    nc.scalar.copy(XTg[:, t, :], pxt[:])

                pxs = ps_s1.tile([D, 1], fp32, tag="small64")
                for t in range(NT):
                    nc.tensor.matmul(pxs[:], Xg[:, t, :], ones_col[:],
                                     start=(t == 0), stop=(t == NT - 1))
                meanX = smpool.tile([D, 1], fp32, tag="meanX")
                nc.scalar.activation(meanX[:], pxs[:], Act.Copy, scale=1.0 / n)

                # smalls: cols 0:8 rs_q | 8:16 diag | 16:24 p1 | 24:32 rowmean
                #         32:40 u | 40 g1 | 41 g2
                S = smpool.tile([P, 48], fp32, tag="S")
                scratchD = smpool.tile([P, D], fp32, tag="scratchD")
                for t in range(NT):
                    nc.vector.scalar_tensor_tensor(
                        out=scratchD[:], in0=Xg[:, t, :], scalar=1.0,
                        in1=wA1_rep[:], op0=Alu.mult, op1=Alu.mult,
                        accum_out=S[:, 16 + t:17 + t])

                # ---- A slabs: load, diag, prescale(+rowsum accum) ----
                scratchP = smpool.tile([P, P], fp32, tag="scratchP")
                slabs = []
                for t in range(NT):
                    sl = spool.tile([P, n], fp32, tag="slab")
                    nc.sync.dma_start(sl[:], A[g, t * P:(t + 1) * P, :])
                    slabs.append(sl)
                    nc.vector.scalar_tensor_tensor(
                        out=scratchP[:], in0=sl[:, t * P:(t + 1) * P],
                        scalar=1.0, in1=ident[:],
                        op0=Alu.mult, op1=Alu.mult,
                        accum_out=S[:, 8 + t:9 + t])
                    if prescale_is_c0:
                        nc.vector.tensor_scalar(
                            out=sl[:], in0=sl[:], scalar1=ap_q, scalar2=None,
                            op0=Alu.mult, op1=Alu.add, accum_out=S[:, t:t + 1])
                    else:
                        # accumulate raw rowsum, then scale by c0 on ACT
                        nc.vector.tensor_scalar(
                            out=sl[:], in0=sl[:], scalar1=1.0, scalar2=None,
                            op0=Alu.mult, op1=Alu.add, accum_out=S[:, t:t + 1])
                        nc.scalar.activation(sl[:], sl[:], Act.Copy, scale=ap_q)

                # ---- PE block transposes of c0*A ----
                ATs = []
                for jt in range(NT):
                    atj = tpool.tile([P, n], fp32, tag="atT")
                    for half in range(2):
                        pst = ps_t.tile([P, 512], fp32, tag="ps_t")
                        for q4 in range(4):
                            it = half * 4 + q4
                            nc.tensor.transpose(
                                pst[:, q4 * P:(q4 + 1) * P],
                                slabs[it][:, jt * P:(jt + 1) * P], ident[:])
                        nc.scalar.copy(atj[:, half * 512:(half + 1) * 512], pst[:])
                    ATs.append(atj)

                # ---- stats cascade ----
                nc.vector.tensor_scalar(out=S[:, 24:32], in0=S[:, 0:8],
                                        scalar1=ap_rinv, scalar2=None, op0=Alu.mult)
                nc.vector.scalar_tensor_tensor(
                    out=S[:, 32:40], in0=S[:, 8:16], scalar=ap_c4,
                    in1=S[:, 16:24], op0=Alu.mult, op1=Alu.add)
                nc.vector.scalar_tensor_tensor(
                    out=S[:, 32:40], in0=S[:, 24:32], scalar=ap_c3,
                    in1=S[:, 32:40], op0=Alu.mult, op1=Alu.add)

                sums2 = smpool.tile([P, 2], fp32, tag="sums2")
                nc.vector.reduce_sum(sums2[:, 0:1], S[:, 24:32], axis=AxisX)
                nc.vector.reduce_sum(sums2[:, 1:2], S[:, 8:16], axis=AxisX)
                # partition-sum via ones^T @ sums2 -> [1,2] row (mean_all, mean_diag)*n
                pmad = ps_s2.tile([1, 2], fp32, tag="misc")
                nc.tensor.matmul(pmad[:], ones_col[:], sums2[:],
                                 start=True, stop=True)
                mad_sb = smpool.tile([1, 2], fp32, tag="mad_sb")
                nc.scalar.activation(mad_sb[:], pmad[:], Act.Copy, scale=1.0 / n)
                # mad_sb = [mean_all, mean_diag] on partition 0

                # p2 = meanX . wA2 -> [1,1]
                pp2 = ps_s1.tile([1, 1], fp32, tag="small64")
                nc.tensor.matmul(pp2[:], meanX[:], wA2_sb[:], start=True, stop=True)

                # s = c1*mean_all + c2*mean_diag + p2  (on partition 0)
                s_row = smpool.tile([1, 1], fp32, tag="s_row")
                nc.vector.scalar_tensor_tensor(
                    out=s_row[:], in0=mad_sb[0:1, 0:1], scalar=ap_c1[0:1, :],
                    in1=pp2[0:1, :], op0=Alu.mult, op1=Alu.add)
                nc.vector.scalar_tensor_tensor(
                    out=s_row[:], in0=mad_sb[0:1, 1:2], scalar=ap_c2[0:1, :],
                    in1=s_row[:], op0=Alu.mult, op1=Alu.add)
                # broadcast s to all partitions: ones_row^T(K=1) @ s_row
                psr = ps_s1.tile([P, 1], fp32, tag="small64")
                nc.tensor.matmul(psr[:], ones_row[:], s_row[:],
                                 start=True, stop=True)
                s_rep = smpool.tile([P, 1], fp32, tag="s_rep")
                nc.scalar.copy(s_rep[:], psr[:])

                uis = smpool.tile([P, NT], fp32, tag="uis")
                nc.vector.tensor_scalar(out=uis[:], in0=S[:, 32:40],
                                        scalar1=s_rep[:, 0:1], scalar2=None,
                                        op0=Alu.add)

                # g1/g2 = wx2 @ meanX + mean_diag*wx5 + mean_all*wx6  (into S cols)
                pmn = ps_s2.tile([D, 2], fp32, tag="misc")
                nc.tensor.matmul(pmn[:], ones_row[0:1, 0:D], mad_sb[:],
                                 start=True, stop=True)
                mn = smpool.tile([D, 2], fp32, tag="mn")
                nc.scalar.copy(mn[:], pmn[:])
                psg = ps_s1.tile([D, 2], fp32, tag="small64")
                nc.tensor.matmul(psg[:, 0:1], wT_sb["w12T"][:], meanX[:],
                                 start=True, stop=True)
                nc.tensor.matmul(psg[:, 1:2], wT_sb["w22T"][:], meanX[:],
                                 start=True, stop=True)
                for k, (w5, w6) in enumerate((("w15c", "w16c"), ("w25c", "w26c"))):
                    col = S[0:D, 40 + k:41 + k]
                    nc.vector.scalar_tensor_tensor(
                        out=col, in0=wc_sb[w5][:], scalar=mn[:, 1:2],
                        in1=psg[:, k:k + 1], op0=Alu.mult, op1=Alu.add)
                    nc.vector.scalar_tensor_tensor(
                        out=col, in0=wc_sb[w6][:], scalar=mn[:, 0:1],
                        in1=col, op0=Alu.mult, op1=Alu.add)

                # ---- bundle transpose -> row layouts ----
                psb = ps_s2.tile([48, P], fp32, tag="misc")
                nc.tensor.transpose(psb[:], S[:, 0:48], ident[:])
                bndT = smpool.tile([48, P], fp32, tag="bndT")
                nc.scalar.copy(bndT[:], psb[:])

                # rows: 0:n rowmean | n:2n diag | 2n:3n u | 3n:+D g1 | +D:+2D g2
                rows = rpool.tile([1, 3 * n + 2 * D], fp32, tag="rows")
                nc.sync.dma_start(rows[0:1, 0:n], bndT[24:32, :])
                nc.sync.dma_start(rows[0:1, n:2 * n], bndT[8:16, :])
                nc.sync.dma_start(rows[0:1, 2 * n:3 * n], bndT[32:40, :])
                nc.sync.dma_start(rows[0:1, 3 * n:3 * n + D], bndT[40:41, 0:D])
                nc.sync.dma_start(rows[0:1, 3 * n + D:3 * n + 2 * D],
                                  bndT[41:42, 0:D])

                urow_rep = upool.tile([P, n], fp32, tag="urep")
                for h in range(2):
                    pur = ps_t.tile([P, 512], fp32, tag="ps_t")
                    nc.tensor.matmul(
                        pur[:], ones_row[:],
                        rows[0:1, 2 * n + h * 512:2 * n + (h + 1) * 512],
                        start=True, stop=True)
                    eng = nc.scalar if h == 0 else nc.vector
                    if h == 0:
                        nc.scalar.copy(urow_rep[:, h * 512:(h + 1) * 512], pur[:])
                    else:
                        nc.vector.tensor_copy(urow_rep[:, h * 512:(h + 1) * 512], pur[:])

                # ---- X1_t / X2_t assembly ----
                X1t = xpool.tile([P, NT, D], fp32, tag="X1t")
                X2t = xpool.tile([P, NT, D], fp32, tag="X2t")
                for t in range(NT):
                    for dst, wT, wa, wb, goff in (
                        (X1t, "w11T", "w13r", "w14r", 3 * n),
                        (X2t, "w21T", "w23r", "w24r", 3 * n + D),
                    ):
                        psx = ps_x.tile([P, D], fp32, tag="ps_x")
                        nc.tensor.matmul(psx[:], XTg[:, t, :], wT_sb[wT][:],
                                         start=True, stop=False)
                        nc.tensor.matmul(psx[:], rows[0:1, t * P:(t + 1) * P],
                                         wr_sb[wa][:], start=False, stop=False)
                        nc.tensor.matmul(psx[:], rows[0:1, n + t * P:n + (t + 1) * P],
                                         wr_sb[wb][:], start=False, stop=False)
                        nc.tensor.matmul(psx[:], ones_row[:],
                                         rows[0:1, goff:goff + D],
                                         start=False, stop=True)
                        nc.scalar.copy(dst[:, t, :], psx[:])

                # ---- colsum2 / uX2 rank-1 rhs rows ----
                pcs = ps_s2.tile([1, 2 * D], fp32, tag="misc")
                for t in range(NT):
                    nc.tensor.matmul(pcs[0:1, 0:D], ones_col[:], X2t[:, t, :],
                                     start=(t == 0), stop=(t == NT - 1))
                for t in range(NT):
                    nc.tensor.matmul(pcs[0:1, D:2 * D], S[:, 32 + t:33 + t],
                                     X2t[:, t, :], start=(t == 0), stop=(t == NT - 1))
                rhs2 = smpool.tile([1, 2 * D], fp32, tag="rhs2")
                nc.scalar.copy(rhs2[0:1, 0:D], pcs[0:1, 0:D])
                nc.vector.scalar_tensor_tensor(
                    out=rhs2[0:1, D:2 * D], in0=rhs2[0:1, 0:D],
                    scalar=s_rep[0:1, 0:1], in1=pcs[0:1, D:2 * D],
                    op0=Alu.mult, op1=Alu.add)
                # fixup: rank-1 rhs must carry q/c0 so eviction scale c0/(q*n)
                # nets 1/n on the rank-1 terms
                nc.vector.tensor_scalar(out=rhs2[:], in0=rhs2[:],
                                        scalar1=ap_qc0, scalar2=None, op0=Alu.mult)

                # ---- A_t finalize + store ----
                for t in range(NT):
                    nc.vector.scalar_tensor_tensor(
                        out=slabs[t][:], in0=slabs[t][:],
                        scalar=uis[:, t:t + 1], in1=urow_rep[:],
                        op0=Alu.add, op1=Alu.add)
                    nc.sync.dma_start(At[g, t * P:(t + 1) * P, :], slabs[t][:])

                # ---- big matmul + eviction ----
                og = opool.tile([P, NT, D], fp32, tag="og")
                for it in range(NT):
                    psm = ps_mm.tile([P, D], fp32, tag="ps_mm")
                    for jt in range(NT):
                        nc.tensor.matmul(psm[:], ATs[jt][:, it * P:(it + 1) * P],
                                         X2t[:, jt, :],
                                         start=(jt == 0), stop=False)
                    nc.tensor.matmul(psm[:],
                                     rows[0:1, 2 * n + it * P:2 * n + (it + 1) * P],
                                     rhs2[0:1, 0:D], start=False, stop=False)
                    nc.tensor.matmul(psm[:], ones_row[:], rhs2[0:1, D:2 * D],
                                     start=False, stop=True)
                    nc.vector.scalar_tensor_tensor(
                        out=og[:, it, :], in0=psm[:], scalar=ap_evs,
                        in1=X1t[:, it, :], op0=Alu.mult, op1=Alu.add)
                nc.sync.dma_start(Out[g].rearrange("(t p) d -> p t d", p=P), og[:])

    nc.compile()
    _BUILD_CACHE[key] = nc
    return nc


def _host_inputs(inputs):
    c = np.asarray(inputs["coeffs"], np.float32)
    c0, c1, c2, c3, c4 = (float(c[i]) for i in range(5))
    prescale_is_c0 = abs(c0) > 1e-4
    q = c0 if prescale_is_c0 else 1.0
    dc_row = np.array([[q, 1.0 / (q * n), c3, c4, c1, c2,
                        c0 / (q * n), q / c0 if c0 != 0.0 else 0.0]], np.float32)
    dc = np.tile(dc_row, (128, 1))
    shared = {
        "dc": dc,
        "wA1r": np.tile(np.asarray(inputs["wA1"], np.float32).reshape(1, D), (128, 1)),
        "wA2c": np.asarray(inputs["wA2"], np.float32).reshape(D, 1),
        "w11T": np.ascontiguousarray(np.asarray(inputs["w11"], np.float32).T),
        "w12T": np.ascontiguousarray(np.asarray(inputs["w12"], np.float32).T),
        "w21T": np.ascontiguousarray(np.asarray(inputs["w21"], np.float32).T),
        "w22T": np.ascontiguousarray(np.asarray(inputs["w22"], np.float32).T),
        "w13r": np.asarray(inputs["w13"], np.float32).reshape(1, D),
        "w14r": np.asarray(inputs["w14"], np.float32).reshape(1, D),
        "w23r": np.asarray(inputs["w23"], np.float32).reshape(1, D),
        "w24r": np.asarray(inputs["w24"], np.float32).reshape(1, D),
        "w15c": np.asarray(inputs["w15"], np.float32).reshape(D, 1),
        "w16c": np.asarray(inputs["w16"], np.float32).reshape(D, 1),
        "w25c": np.asarray(inputs["w25"], np.float32).reshape(D, 1),
        "w26c": np.asarray(inputs["w26"], np.float32).reshape(D, 1),
    }
    return prescale_is_c0, shared


def kernel(**inputs):
    from concourse.bass_utils import run_bass_kernel_spmd

    prescale_is_c0, shared = _host_inputs(inputs)
    nc = _build(prescale_is_c0)

    A = np.ascontiguousarray(np.asarray(inputs["A"], np.float32))
    X = np.ascontiguousarray(np.asarray(inputs["X"], np.float32))
    in_maps = []
    for k in range(NCORES):
        m = dict(shared)
        m["A"] = np.ascontiguousarray(A[k * G:(k + 1) * G])
        m["X"] = np.ascontiguousarray(X[k * G:(k + 1) * G])
        in_maps.append(m)

    res = run_bass_kernel_spmd(nc, in_maps, core_ids=list(range(NCORES)))
    At = np.concatenate([r["At"] for r in res.results], axis=0)
    Out = np.concatenate([r["Out"] for r in res.results], axis=0)
    return At, Out


if __name__ == "__main__":
    import reference

    ins = {k: np.asarray(v) for k, v in reference.setup_inputs().items()}
    got = kernel(**ins)
    exp = reference.reference(**ins)
    for name, g_, e_ in zip(("At", "Out"), got, exp):
        e_ = np.asarray(e_)
        err = np.abs(g_ - e_).max() / max(1e-12, np.abs(e_).max())
        print(name, g_.shape, "rel err:", err)


# revision 15
# speedup vs baseline: 1.2384x; 1.2384x over previous
"""Trainium2 Bass kernel for nn_GNN_layer (gnn_message_passing).

Host contract: kernel(**inputs) takes the FULL unsharded inputs (as produced
by setup_inputs) and returns the full (A_t, out) tuple, running the Bass
kernel data-parallel over the graph batch dim on 8 NeuronCores.

Device strategy per core (2 graphs):
  - load A row-slabs [128,1024]; fused DVE prescale (c0*A, in place) with
    free-axis accum -> c0*rowsum; fused mult-with-identity reduce -> diag
  - PE block-transposes of c0*A (batched 4 blocks/PSUM bank, ACT copy out)
  - stats cascade -> u = c3*rowmean + c4*diag + p1, s = c1*mean_all +
    c2*mean_diag + p2 (p1/p2/meanX via tiny PE/DVE ops on X)
  - A_t = (c0*A + (u_i+s)) + u_j : one fused DVE scalar_tensor_tensor per slab
  - out = (A_t@X2_t)/n + X1_t via decomposition:
      A_t@X2 = c0*(A@X2) + u (x) colsum2 + 1 (x) (u^T X2 + s*colsum2)
    PE does c0A@X2 with transposed blocks; the rank-1 terms are K=1 matmuls
    accumulated into the same PSUM group.
"""

import os
import sys

import numpy as np

for _p in ("/opt/trn_rl_repo", os.path.expanduser("~/.axon_site/_ro/trn_rl_repo")):
    if os.path.isdir(_p) and _p not in sys.path:
        sys.path.insert(0, _p)

N_FULL, n, D = 16, 1024, 64
NCORES = 8
G = N_FULL // NCORES  # graphs per core
P = 128
NT = n // P  # 8 row tiles per graph

_BUILD_CACHE = {}


def _build(prescale_is_c0: bool, stage: int = 99):
    """Build the per-core Bass program. prescale_is_c0: if True the slab
    prescale multiplies by c0 (dc[0]) and rowmean is recovered via 1/(c0*n);
    if False the prescale is by 1.0 (used when |c0| is tiny)."""
    key = (prescale_is_c0, stage)
    if key in _BUILD_CACHE:
        return _BUILD_CACHE[key]

    import concourse.bass as bass
    import concourse.mybir as mybir
    from concourse import bacc, tile
    from concourse.masks import make_identity

    fp32 = mybir.dt.float32
    Alu = mybir.AluOpType
    Act = mybir.ActivationFunctionType
    AxisX = mybir.AxisListType.X
    from concourse import bass_isa

    nc = bacc.Bacc("TRN2", target_bir_lowering=False)

    A = nc.dram_tensor("A", [G, n, n], fp32, kind="ExternalInput")
    X = nc.dram_tensor("X", [G, n, D], fp32, kind="ExternalInput")
    dc = nc.dram_tensor("dc", [128, 8], fp32, kind="ExternalInput")
    wA1r = nc.dram_tensor("wA1r", [128, D], fp32, kind="ExternalInput")
    wA2c = nc.dram_tensor("wA2c", [D, 1], fp32, kind="ExternalInput")
    w11T = nc.dram_tensor("w11T", [D, D], fp32, kind="ExternalInput")
    w12T = nc.dram_tensor("w12T", [D, D], fp32, kind="ExternalInput")
    w21T = nc.dram_tensor("w21T", [D, D], fp32, kind="ExternalInput")
    w22T = nc.dram_tensor("w22T", [D, D], fp32, kind="ExternalInput")
    w13r = nc.dram_tensor("w13r", [1, D], fp32, kind="ExternalInput")
    w14r = nc.dram_tensor("w14r", [1, D], fp32, kind="ExternalInput")
    w23r = nc.dram_tensor("w23r", [1, D], fp32, kind="ExternalInput")
    w24r = nc.dram_tensor("w24r", [1, D], fp32, kind="ExternalInput")
    w15c = nc.dram_tensor("w15c", [D, 1], fp32, kind="ExternalInput")
    w16c = nc.dram_tensor("w16c", [D, 1], fp32, kind="ExternalInput")
    w25c = nc.dram_tensor("w25c", [D, 1], fp32, kind="ExternalInput")
    w26c = nc.dram_tensor("w26c", [D, 1], fp32, kind="ExternalInput")

    onesd = nc.dram_tensor("onesd", [1, n], fp32, kind="ExternalInput")

    At = nc.dram_tensor("At", [G, n, n], fp32, kind="ExternalOutput")
    Out = nc.dram_tensor("Out", [G, n, D], fp32, kind="ExternalOutput")

    with tile.TileContext(nc) as tc:
        with (
            tc.tile_pool(name="const", bufs=1) as cpool,
            tc.tile_pool(name="slab", bufs=2 * NT) as spool,
            tc.tile_pool(name="atT", bufs=2 * NT) as tpool,
            tc.tile_pool(name="xbuf", bufs=2) as xpool,
            tc.tile_pool(name="small", bufs=2) as smpool,
            tc.tile_pool(name="rows", bufs=2) as rpool,
            tc.tile_pool(name="urep", bufs=2) as upool,
            tc.tile_pool(name="obuf", bufs=2) as opool,
            tc.tile_pool(name="ps_t", bufs=2, space="PSUM") as ps_t,
            tc.tile_pool(name="ps_mm", bufs=1, space="PSUM") as ps_mm,
            tc.tile_pool(name="ps_x", bufs=2, space="PSUM") as ps_x,
            tc.tile_pool(name="ps_s1", bufs=1, space="PSUM") as ps_s1,
            tc.tile_pool(name="ps_s2", bufs=1, space="PSUM") as ps_s2,
        ):
            # ---------------- one-time constants ----------------
            ident = cpool.tile([P, P], fp32, tag="ident")
            make_identity(nc, ident[:])
            ones_col = cpool.tile([P, 1], fp32, tag="ones_col")
            nc.vector.memset(ones_col[:], 1.0)

            ones_rn = cpool.tile([1, n], fp32, tag="ones_rn")
            nc.sync.dma_start(ones_rn[:], onesd[:])
            dc_rep = cpool.tile([P, 8], fp32, tag="dc_rep")
            nc.sync.dma_start(dc_rep[:], dc[:])
            ap_q = dc_rep[:, 0:1]        # prescale factor q (c0 or 1)
            ap_rinv = dc_rep[:, 1:2]     # 1/(q*n)
            ap_c3 = dc_rep[:, 2:3]
            ap_c4 = dc_rep[:, 3:4]
            ap_c1 = dc_rep[:, 4:5]       # c1
            ap_c2 = dc_rep[:, 5:6]       # c2
            ap_evs = dc_rep[:, 6:7]      # c0/(q*n)  (eviction scale)
            ap_qc0 = dc_rep[0:1, 7:8]    # q/c0      (rank-1 rhs fixup)

            wA1_rep = cpool.tile([P, D], fp32, tag="wA1_rep")
            nc.sync.dma_start(wA1_rep[:], wA1r[:])

            wA2_sb = cpool.tile([D, 1], fp32, tag="wA2_sb")
            nc.sync.dma_start(wA2_sb[:], wA2c[:])
            wT_sb = {}
            for nm, t_ in (("w11T", w11T), ("w12T", w12T), ("w21T", w21T), ("w22T", w22T)):
                s_ = cpool.tile([D, D], fp32, tag=nm)
                nc.sync.dma_start(s_[:], t_[:])
                wT_sb[nm] = s_
            wr_sb = {}
            for nm, t_ in (("w13r", w13r), ("w14r", w14r), ("w23r", w23r), ("w24r", w24r)):
                s_ = cpool.tile([1, D], fp32, tag=nm)
                nc.sync.dma_start(s_[:], t_[:])
                wr_sb[nm] = s_
            wc_sb = {}
            for nm, t_ in (("w15c", w15c), ("w16c", w16c), ("w25c", w25c), ("w26c", w26c)):
                s_ = cpool.tile([D, 1], fp32, tag=nm)
                nc.sync.dma_start(s_[:], t_[:])
                wc_sb[nm] = s_

            # PE sync pre-warm: first PE instruction depends only on ident
            # (Pool sem) so later matmuls carry at most one new sync wait.
            dummy_ps = ps_s1.tile([P, P], fp32, tag="small64")
            nc.tensor.transpose(dummy_ps[:], ident[:], ident[:])

            # ---------------- per-graph program ----------------
            for g in range(G):
                # ---- X load, transposes, Xsum, p1 ----
                Xg = xpool.tile([P, NT, D], fp32, tag="Xg")
                nc.sync.dma_start(Xg[:], X[g].rearrange("(t p) d -> p t d", p=P))

                XTg = xpool.tile([D, NT, P], fp32, tag="XTg")
                for t in range(NT):
                    pxt = ps_x.tile([D, P], fp32, tag="ps_x")
                    nc.tensor.transpose(pxt[:], Xg[:, t, :], ident[:])
                    nc.scalar.copy(XTg[:, t, :], pxt[:])

                pxs = ps_s1.tile([D, 1], fp32, tag="small64")
                for t in range(NT):
                    nc.tensor.matmul(pxs[:], Xg[:, t, :], ones_col[:],
                                     start=(t == 0), stop=(t == NT - 1))
                meanX = smpool.tile([D, 1], fp32, tag="meanX")
                nc.scalar.activation(meanX[:], pxs[:], Act.Copy, scale=1.0 / n)

                # smalls: cols 0:8 rs_q | 8:16 diag | 16:24 p1 | 24:32 rowmean
                #         32:40 u | 40 g1 | 41 g2
                S = smpool.tile([P, 48], fp32, tag="S")
                scratchD = smpool.tile([P, D], fp32, tag="scratchD")
                for t in range(NT):
                    nc.vector.scalar_tensor_tensor(
                        out=scratchD[:], in0=Xg[:, t, :], scalar=1.0,
                        in1=wA1_rep[:], op0=Alu.mult, op1=Alu.mult,
                        accum_out=S[:, 16 + t:17 + t])

                # ---- A slabs: load, diag, prescale(+rowsum accum) ----
                scratchP = smpool.tile([P, P], fp32, tag="scratchP")
                slabs = []
                for t in range(NT):
                    sl = spool.tile([P, n], fp32, tag="slab")
                    nc.sync.dma_start(sl[:], A[g, t * P:(t + 1) * P, :])
                    slabs.append(sl)
                    nc.vector.scalar_tensor_tensor(
                        out=scratchP[:], in0=sl[:, t * P:(t + 1) * P],
                        scalar=1.0, in1=ident[:],
                        op0=Alu.mult, op1=Alu.mult,
                        accum_out=S[:, 8 + t:9 + t])
                    if prescale_is_c0:
                        nc.vector.tensor_scalar(
                            out=sl[:], in0=sl[:], scalar1=ap_q, scalar2=None,
                            op0=Alu.mult, op1=Alu.add, accum_out=S[:, t:t + 1])
                    else:
                        # accumulate raw rowsum, then scale by c0 on ACT
                        nc.vector.tensor_scalar(
                            out=sl[:], in0=sl[:], scalar1=1.0, scalar2=None,
                            op0=Alu.mult, op1=Alu.add, accum_out=S[:, t:t + 1])
                        nc.scalar.activation(sl[:], sl[:], Act.Copy, scale=ap_q)

                # ---- PE block transposes of c0*A ----
                ATs = []
                for jt in range(NT):
                    atj = tpool.tile([P, n], fp32, tag="atT")
                    for half in range(2):
                        pst = ps_t.tile([P, 512], fp32, tag="ps_t")
                        for q4 in range(4):
                            it = half * 4 + q4
                            nc.tensor.transpose(
                                pst[:, q4 * P:(q4 + 1) * P],
                                slabs[it][:, jt * P:(jt + 1) * P], ident[:])
                        nc.scalar.copy(atj[:, half * 512:(half + 1) * 512], pst[:])
                    ATs.append(atj)

                # ---- stats cascade ----
                nc.vector.tensor_scalar(out=S[:, 24:32], in0=S[:, 0:8],
                                        scalar1=ap_rinv, scalar2=None, op0=Alu.mult)
                nc.vector.scalar_tensor_tensor(
                    out=S[:, 32:40], in0=S[:, 8:16], scalar=ap_c4,
                    in1=S[:, 16:24], op0=Alu.mult, op1=Alu.add)
                nc.vector.scalar_tensor_tensor(
                    out=S[:, 32:40], in0=S[:, 24:32], scalar=ap_c3,
                    in1=S[:, 32:40], op0=Alu.mult, op1=Alu.add)

                sums2 = smpool.tile([P, 2], fp32, tag="sums2")
                nc.vector.reduce_sum(sums2[:, 0:1], S[:, 24:32], axis=AxisX)
                nc.vector.reduce_sum(sums2[:, 1:2], S[:, 8:16], axis=AxisX)
                # partition-sum via ones^T @ sums2 -> [1,2] row (mean_all, mean_diag)*n
                pmad = ps_s2.tile([1, 2], fp32, tag="misc")
                nc.tensor.matmul(pmad[:], ones_col[:], sums2[:],
                                 start=True, stop=True)
                mad_sb = smpool.tile([1, 2], fp32, tag="mad_sb")
                nc.scalar.activation(mad_sb[:], pmad[:], Act.Copy, scale=1.0 / n)
                # mad_sb = [mean_all, mean_diag] on partition 0

                # p2 = meanX . wA2 -> [1,1]
                pp2 = ps_s1.tile([1, 1], fp32, tag="small64")
                nc.tensor.matmul(pp2[:], meanX[:], wA2_sb[:], start=True, stop=True)

                # s = c1*mean_all + c2*mean_diag + p2  (on partition 0)
                s_row = smpool.tile([1, 1], fp32, tag="s_row")
                nc.vector.scalar_tensor_tensor(
                    out=s_row[:], in0=mad_sb[0:1, 0:1], scalar=ap_c1[0:1, :],
                    in1=pp2[0:1, :], op0=Alu.mult, op1=Alu.add)
                nc.vector.scalar_tensor_tensor(
                    out=s_row[:], in0=mad_sb[0:1, 1:2], scalar=ap_c2[0:1, :],
                    in1=s_row[:], op0=Alu.mult, op1=Alu.add)
                # broadcast s to all partitions: ones_row^T(K=1) @ s_row
                psr = ps_s1.tile([P, 1], fp32, tag="small64")
                nc.tensor.matmul(psr[:], ones_rn[0:1, 0:P], s_row[:],
                                 start=True, stop=True)
                s_rep = smpool.tile([P, 1], fp32, tag="s_rep")
                nc.scalar.copy(s_rep[:], psr[:])

                uis = smpool.tile([P, NT], fp32, tag="uis")
                nc.vector.tensor_scalar(out=uis[:], in0=S[:, 32:40],
                                        scalar1=s_rep[:, 0:1], scalar2=None,
                                        op0=Alu.add)

                # g1/g2 = wx2 @ meanX + mean_diag*wx5 + mean_all*wx6  (into S cols)
                pmn = ps_s2.tile([D, 2], fp32, tag="misc")
                nc.tensor.matmul(pmn[:], ones_rn[0:1, 0:D], mad_sb[:],
                                 start=True, stop=True)
                mn = smpool.tile([D, 2], fp32, tag="mn")
                nc.scalar.copy(mn[:], pmn[:])
                psg = ps_s1.tile([D, 2], fp32, tag="small64")
                nc.tensor.matmul(psg[:, 0:1], wT_sb["w12T"][:], meanX[:],
                                 start=True, stop=True)
                nc.tensor.matmul(psg[:, 1:2], wT_sb["w22T"][:], meanX[:],
                                 start=True, stop=True)
                for k, (w5, w6) in enumerate((("w15c", "w16c"), ("w25c", "w26c"))):
                    col = S[0:D, 40 + k:41 + k]
                    nc.vector.scalar_tensor_tensor(
                        out=col, in0=wc_sb[w5][:], scalar=mn[:, 1:2],
                        in1=psg[:, k:k + 1], op0=Alu.mult, op1=Alu.add)
                    nc.vector.scalar_tensor_tensor(
                        out=col, in0=wc_sb[w6][:], scalar=mn[:, 0:1],
                        in1=col, op0=Alu.mult, op1=Alu.add)

                # ---- bundle transpose -> row layouts ----
                psb = ps_s2.tile([48, P], fp32, tag="misc")
                nc.tensor.transpose(psb[:], S[:, 0:48], ident[:])
                bndT = smpool.tile([48, P], fp32, tag="bndT")
                nc.scalar.copy(bndT[:], psb[:])

                # stack3: p0 rowmean | p1 diag | p2 ones  (K=3 lhsT for X-asm)
                stack3 = rpool.tile([3, n], fp32, tag="stack3")
                nc.sync.dma_start(stack3[0:1, :], bndT[24:32, :])
                nc.sync.dma_start(stack3[1:2, :], bndT[8:16, :])
                nc.sync.dma_start(stack3[2:3, :], onesd[:])
                # u as a row (urow_rep source + MM2 rank-1 rhs)
                urow = rpool.tile([1, n], fp32, tag="urow")
                nc.sync.dma_start(urow[0:1, :], bndT[32:40, :])
                # xrhs: p0 w13/w23 | p1 w14/w24 | p2 g1/g2  (K=3 rhs for X-asm)
                xrhs1 = rpool.tile([3, D], fp32, tag="xrhs1")
                nc.sync.dma_start(xrhs1[0:1, :], w13r[:])
                nc.sync.dma_start(xrhs1[1:2, :], w14r[:])
                nc.sync.dma_start(xrhs1[2:3, :], bndT[40:41, 0:D])
                xrhs2 = rpool.tile([3, D], fp32, tag="xrhs2")
                nc.sync.dma_start(xrhs2[0:1, :], w23r[:])
                nc.sync.dma_start(xrhs2[1:2, :], w24r[:])
                nc.sync.dma_start(xrhs2[2:3, :], bndT[41:42, 0:D])

                urow_rep = upool.tile([P, n], fp32, tag="urep")
                for h in range(2):
                    pur = ps_t.tile([P, 512], fp32, tag="ps_t")
                    nc.tensor.matmul(
                        pur[:], ones_rn[0:1, 0:P],
                        urow[0:1, h * 512:(h + 1) * 512],
                        start=True, stop=True)
                    if h == 0:
                        nc.scalar.copy(urow_rep[:, h * 512:(h + 1) * 512], pur[:])
                    else:
                        nc.vector.tensor_copy(urow_rep[:, h * 512:(h + 1) * 512], pur[:])

                # ---- X1_t / X2_t assembly ----
                X1t = xpool.tile([P, NT, D], fp32, tag="X1t")
                X2t = xpool.tile([P, NT, D], fp32, tag="X2t")
                for t in range(NT):
                    for dst, wT, xrhs in ((X1t, "w11T", xrhs1),
                                          (X2t, "w21T", xrhs2)):
                        psx = ps_x.tile([P, D], fp32, tag="ps_x")
                        nc.tensor.matmul(psx[:], XTg[:, t, :], wT_sb[wT][:],
                                         start=True, stop=False)
                        nc.tensor.matmul(psx[:], stack3[:, t * P:(t + 1) * P],
                                         xrhs[:], start=False, stop=True)
                        nc.scalar.copy(dst[:, t, :], psx[:])

                # ---- colsum2 / uX2 rank-1 rhs rows ----
                pcs = ps_s2.tile([1, 2 * D], fp32, tag="misc")
                for t in range(NT):
                    nc.tensor.matmul(pcs[0:1, 0:D], ones_col[:], X2t[:, t, :],
                                     start=(t == 0), stop=(t == NT - 1))
                for t in range(NT):
                    nc.tensor.matmul(pcs[0:1, D:2 * D], S[:, 32 + t:33 + t],
                                     X2t[:, t, :], start=(t == 0), stop=(t == NT - 1))
                rhs2 = smpool.tile([1, 2 * D], fp32, tag="rhs2")
                nc.scalar.copy(rhs2[0:1, 0:D], pcs[0:1, 0:D])
                nc.vector.scalar_tensor_tensor(
                    out=rhs2[0:1, D:2 * D], in0=rhs2[0:1, 0:D],
                    scalar=s_rep[0:1, 0:1], in1=pcs[0:1, D:2 * D],
                    op0=Alu.mult, op1=Alu.add)
                # fixup: rank-1 rhs must carry q/c0 so eviction scale c0/(q*n)
                # nets 1/n on the rank-1 terms
                nc.vector.tensor_scalar(out=rhs2[:], in0=rhs2[:],
                                        scalar1=ap_qc0, scalar2=None, op0=Alu.mult)

                # ---- A_t finalize + store ----
                for t in range(NT):
                    nc.vector.scalar_tensor_tensor(
                        out=slabs[t][:], in0=slabs[t][:],
                        scalar=uis[:, t:t + 1], in1=urow_rep[:],
                        op0=Alu.add, op1=Alu.add)
                    nc.sync.dma_start(At[g, t * P:(t + 1) * P, :], slabs[t][:])

                # ---- big matmul (transposed accumulation, N=512) + eviction ----
                # potile[e, it, i] accumulates (A@X2)^T; X2 blocks are the
                # stationary operand (64-col weight loads) and the transposed
                # A blocks stream 512 wide, four i-blocks per matmul. The two
                # rank-1 correction terms ride the same accumulation group.
                og = opool.tile([P, NT, D], fp32, tag="og")
                oT = opool.tile([D, NT, P], fp32, tag="oT")
                potile = ps_mm.tile([D, NT, P], fp32, tag="ps_mm")
                H = NT // 2 * P  # 512
                for half in range(2):
                    pot_h = potile[:, half * 4:(half + 1) * 4, :]
                    for jt in range(NT):
                        nc.tensor.matmul(pot_h, X2t[:, jt, :],
                                         ATs[jt][:, half * H:(half + 1) * H],
                                         start=(jt == 0), stop=False)
                    nc.tensor.matmul(pot_h, rhs2[0:1, 0:D],
                                     urow[0:1, half * H:(half + 1) * H],
                                     start=False, stop=False)
                    nc.tensor.matmul(pot_h, rhs2[0:1, D:2 * D],
                                     ones_rn[0:1, half * H:(half + 1) * H],
                                     start=False, stop=True)
                    nc.scalar.copy(oT[:, half * 4:(half + 1) * 4, :], pot_h)
                for it in range(NT):
                    pT2 = ps_x.tile([P, D], fp32, tag="ps_x")
                    nc.tensor.transpose(pT2[:], oT[:, it, :], ident[0:D, 0:D])
                    nc.vector.scalar_tensor_tensor(
                        out=og[:, it, :], in0=pT2[:], scalar=ap_evs,
                        in1=X1t[:, it, :], op0=Alu.mult, op1=Alu.add)
                nc.sync.dma_start(Out[g].rearrange("(t p) d -> p t d", p=P), og[:])

    nc.compile()
    _BUILD_CACHE[key] = nc
    return nc


def _host_inputs(inputs):
    c = np.asarray(inputs["coeffs"], np.float32)
    c0, c1, c2, c3, c4 = (float(c[i]) for i in range(5))
    prescale_is_c0 = abs(c0) > 1e-4
    q = c0 if prescale_is_c0 else 1.0
    dc_row = np.array([[q, 1.0 / (q * n), c3, c4, c1, c2,
                        c0 / (q * n), q / c0 if c0 != 0.0 else 0.0]], np.float32)
    dc = np.tile(dc_row, (128, 1))
    shared = {
        "dc": dc,
        "onesd": np.ones((1, n), np.float32),
        "wA1r": np.tile(np.asarray(inputs["wA1"], np.float32).reshape(1, D), (128, 1)),
        "wA2c": np.asarray(inputs["wA2"], np.float32).reshape(D, 1),
        "w11T": np.ascontiguousarray(np.asarray(inputs["w11"], np.float32).T),
        "w12T": np.ascontiguousarray(np.asarray(inputs["w12"], np.float32).T),
        "w21T": np.ascontiguousarray(np.asarray(inputs["w21"], np.float32).T),
        "w22T": np.ascontiguousarray(np.asarray(inputs["w22"], np.float32).T),
        "w13r": np.asarray(inputs["w13"], np.float32).reshape(1, D),
        "w14r": np.asarray(inputs["w14"], np.float32).reshape(1, D),
        "w23r": np.asarray(inputs["w23"], np.float32).reshape(1, D),
        "w24r": np.asarray(inputs["w24"], np.float32).reshape(1, D),
        "w15c": np.asarray(inputs["w15"], np.float32).reshape(D, 1),
        "w16c": np.asarray(inputs["w16"], np.float32).reshape(D, 1),
        "w25c": np.asarray(inputs["w25"], np.float32).reshape(D, 1),
        "w26c": np.asarray(inputs["w26"], np.float32).reshape(D, 1),
    }
    return prescale_is_c0, shared


def kernel(**inputs):
    from concourse.bass_utils import run_bass_kernel_spmd

    prescale_is_c0, shared = _host_inputs(inputs)
    nc = _build(prescale_is_c0)

    A = np.ascontiguousarray(np.asarray(inputs["A"], np.float32))
    X = np.ascontiguousarray(np.asarray(inputs["X"], np.float32))
    in_maps = []
    for k in range(NCORES):
        m = dict(shared)
        m["A"] = np.ascontiguousarray(A[k * G:(k + 1) * G])
        m["X"] = np.ascontiguousarray(X[k * G:(k + 1) * G])
        in_maps.append(m)

    res = run_bass_kernel_spmd(nc, in_maps, core_ids=list(range(NCORES)))
    At = np.concatenate([r["At"] for r in res.results], axis=0)
    Out = np.concatenate([r["Out"] for r in res.results], axis=0)
    return At, Out


if __name__ == "__main__":
    import reference

    ins = {k: np.asarray(v) for k, v in reference.setup_inputs().items()}
    got = kernel(**ins)
    exp = reference.reference(**ins)
    for name, g_, e_ in zip(("At", "Out"), got, exp):
        e_ = np.asarray(e_)
        err = np.abs(g_ - e_).max() / max(1e-12, np.abs(e_).max())
        print(name, g_.shape, "rel err:", err)
